# revision 8
# baseline (speedup 1.0000x reference)
"""BinaryLinear Trainium2 kernel (v5 — all-fp8 DoubleRow, hi/lo + packed).

Computes out = x @ sign(weight).T + bias for x [8192, 4096] f32,
weight [4096, 4096] f32, bias [4096] f32.

Strategy: data-parallel over the token dim across 8 NeuronCores
(1024 tokens per core, weight/bias replicated, no collectives).

v5 rationale: every matmul runs in fp8e4 DoubleRow mode (HW-measured
~228ns per K=256xM128xN512 MM here vs 253ns for a bf16 K=128 MM).
The binary weights are exact in fp8; only x quantization loses
precision, so the contraction splits into two zones:
  zone A (first N_A k-tiles): DR slots carry (e4m3(x), e4m3(residual))
    with sign(w) duplicated in both weight slots — one k-tile per MM
    at ~bf16 accuracy (residual recovers the quantization error).
  zone B (remaining k-tiles): standard DR packing, two k-tiles per MM,
    plain e4m3 x (2.64% stand-alone error, diluted by the split).
N_A=18 gives rel err 1.75% (verified exactly against the fixed
reference inputs; gate is 2e-2) at a ~365us MM floor
(16 waves x 4 x (N_A + 7) MMs x 228ns).

Per-core pipeline (host hands xT [4096i, 1024t], wT [4096i, 4096o]):
  1. x zone A: bf16 cast-load [128i, 1024t] (transient), then ScalarE
     copy -> fp8 hi slot and DVE subtract -> fp8 residual slot of
     xa [128i, 2, 1024t]. x zone B: direct SWDGE cast-load f32->fp8
     pairs [128i, 2slot, 1024t].
  2. w per o-quarter: bf16 slab cast-loads; ScalarE sign (scale=1e30
     keeps sign(0)=0; sign must see bf16, not fp8 — casting raw w to
     fp8 first would flush |w| < 2^-9 to zero) writes fp8 slots:
     zone A slot0 via ACT, slot1 duplicated via DVE copy; zone B the
     two packed k-tiles' signs.
  3. TensorE per o-pair wave: N_A zone-A DR MMs + 7 zone-B DR MMs per
     (o-tile, t-half), accumulating f32 into PSUM banks.
  4. DVE tensor_scalar adds the per-partition bias while copying
     PSUM->SBUF; SWDGE stores rows of out_T [4096, 1024].
Host: gather + transpose out_T back to [tokens, 4096].
"""

import numpy as np

import concourse.mybir as mybir
import concourse.tile as tile
from concourse import bacc
from concourse.bass import ts

P = 128
TOKENS, IN_F, OUT_F = 8192, 4096, 4096
N_CORES = 8
OQ = 1024              # output-feature quarter resident in SBUF
N_A = 18               # hi/lo k-tiles; rest packed-pair fp8

F32 = mybir.dt.float32
BF16 = mybir.dt.bfloat16
FP8 = mybir.dt.float8e4
DR = mybir.MatmulPerfMode.DoubleRow


def build_nc(t_shard=TOKENS // N_CORES, in_f=IN_F, out_f=OUT_F, repeat=1,
             n_bf=N_A):
    n_a = n_bf
    ko_tiles = in_f // P        # k tiles of 128
    n_dr = (ko_tiles - n_a) // 2
    assert n_a + 2 * n_dr == ko_tiles and n_a % 2 == 0
    oq_tiles = out_f // OQ      # resident output quarters
    n_per_oq = OQ // P          # 128-wide output tiles per quarter
    t_half = t_shard // 2       # moving-stream length per matmul

    nc = bacc.Bacc(None, target_bir_lowering=False, debug=False)

    xT = nc.dram_tensor("x", [in_f, t_shard], F32, kind="ExternalInput")
    wT = nc.dram_tensor("weight", [in_f, out_f], F32, kind="ExternalInput")
    b = nc.dram_tensor("bias", [out_f], F32, kind="ExternalInput")
    # transposed output: rows = out features, cols = this core's tokens
    out = nc.dram_tensor("out", [out_f, t_shard], F32, kind="ExternalOutput")

    with tile.TileContext(nc) as tc:
        with (
            tc.tile_pool(name="xa", bufs=max(n_a, 1)) as xa_pool,
            tc.tile_pool(name="xq", bufs=max(n_dr, 1)) as xq_pool,
            tc.tile_pool(name="wa", bufs=n_a + 6) as wa_pool,
            tc.tile_pool(name="wq", bufs=n_dr + 3) as wq_pool,
            tc.tile_pool(name="stage", bufs=6) as stage_pool,
            tc.tile_pool(name="bias", bufs=32) as bias_pool,
            tc.tile_pool(name="out_sb", bufs=4) as out_pool,
            tc.tile_pool(name="ps", bufs=8, space="PSUM") as psum_pool,
        ):
          for _rep in range(repeat):

            def emit_w_a(oq, k):
                """Zone-A w tile [128i, 2, OQ] fp8, sign duplicated."""
                wa_k = wa_pool.tile(
                    [P, 2, OQ], FP8, name=f"wa_{oq}_{k}", tag="wa"
                )
                slab = stage_pool.tile(
                    [P, OQ], BF16, name=f"was_{oq}_{k}", tag="stage"
                )
                nc.gpsimd.dma_start(slab, wT[ts(k, P), ts(oq, OQ)])
                nc.scalar.activation(
                    wa_k[:, 0, :], slab,
                    mybir.ActivationFunctionType.Sign, scale=1.0e30,
                )
                nc.vector.tensor_copy(wa_k[:, 1, :], wa_k[:, 0, :])
                return wa_k

            def emit_w_dr(oq, kk):
                """Zone-B w pair-tile [128i, 2, OQ] fp8 (packed k-tiles)."""
                wq_k = wq_pool.tile(
                    [P, 2, OQ], FP8, name=f"wq_{oq}_{kk}", tag="wq"
                )
                for s in range(2):
                    k = n_a + 2 * kk + s
                    slab = stage_pool.tile(
                        [P, OQ], BF16, name=f"ws_{oq}_{kk}_{s}", tag="stage"
                    )
                    nc.gpsimd.dma_start(slab, wT[ts(k, P), ts(oq, OQ)])
                    nc.scalar.activation(
                        wq_k[:, s, :], slab,
                        mybir.ActivationFunctionType.Sign, scale=1.0e30,
                    )
                return wq_k

            def emit_x_a(k):
                """Zone-A x tile [128i, 2, 1024t] fp8: (hi, residual)."""
                xb_k = stage_pool.tile(
                    [P, t_shard], BF16, name=f"xb_{k}", tag="stage"
                )
                nc.gpsimd.dma_start(xb_k, xT[ts(k, P), :])
                xa_k = xa_pool.tile(
                    [P, 2, t_shard], FP8, name=f"xa_{k}", tag="xa"
                )
                nc.scalar.activation(
                    xa_k[:, 0, :], xb_k,
                    mybir.ActivationFunctionType.Copy,
                )
                nc.vector.tensor_tensor(
                    xa_k[:, 1, :], xb_k, xa_k[:, 0, :],
                    mybir.AluOpType.subtract,
                )
                return xa_k

            def emit_x_dr(kk):
                xq_k = xq_pool.tile(
                    [P, 2, t_shard], FP8, name=f"xq_{kk}", tag="xq"
                )
                src = xT[ts(n_a // 2 + kk, 2 * P), :].rearrange(
                    "(s p) t -> p s t", s=2
                )
                nc.gpsimd.dma_start(xq_k, src)
                return xq_k

            # ---- head: x and first-quarter w interleaved in k order so
            # the k=0 matmuls start after a few small loads.
            xa, xq = {}, {}
            wa = {0: [emit_w_a(0, 0)]}
            wq = {0: []}
            for k in range(ko_tiles):
                if k < n_a:
                    xa[k] = emit_x_a(k)
                elif (k - n_a) % 2 == 0:
                    xq[(k - n_a) // 2] = emit_x_dr((k - n_a) // 2)
                if k + 1 < n_a:
                    wa[0].append(emit_w_a(0, k + 1))
                elif k + 1 >= n_a and (k + 1 - n_a) % 2 == 0 and k + 1 < ko_tiles:
                    wq[0].append(emit_w_dr(0, (k + 1 - n_a) // 2))

            bias_t = {}
            for oq in range(oq_tiles):
                for n in range(n_per_oq):
                    bias_n = bias_pool.tile(
                        [P, 1], F32, name=f"b_{oq}_{n}", tag="bias"
                    )
                    nc.gpsimd.dma_start(bias_n, b[ts(oq * n_per_oq + n, P), None])
                    bias_t[(oq, n)] = bias_n

            # ---- main loop: o-quarters, waves of 2 output tiles
            for oq in range(oq_tiles):
                if oq + 1 < oq_tiles:
                    wa[oq + 1] = [emit_w_a(oq + 1, k) for k in range(n_a)]
                    wq[oq + 1] = [emit_w_dr(oq + 1, kk) for kk in range(n_dr)]
                was = wa.pop(oq)
                wqs = wq.pop(oq)

                for wave in range(n_per_oq // 2):
                    psums = {}
                    for wn in range(2):
                        for h in range(2):
                            psums[(wn, h)] = psum_pool.tile(
                                [P, t_half], F32,
                                name=f"ps_{oq}_{wave}_{wn}_{h}", tag="ps",
                            )
                    for k in range(n_a):
                        for wn in range(2):
                            n = wave * 2 + wn
                            for h in range(2):
                                nc.tensor.matmul(
                                    psums[(wn, h)],
                                    was[k][:, :, ts(n, P)],
                                    xa[k][:, :, ts(h, t_half)],
                                    start=(k == 0),
                                    stop=(n_dr == 0 and k == n_a - 1),
                                    perf_mode=DR,
                                )
                    for kk in range(n_dr):
                        for wn in range(2):
                            n = wave * 2 + wn
                            for h in range(2):
                                nc.tensor.matmul(
                                    psums[(wn, h)],
                                    wqs[kk][:, :, ts(n, P)],
                                    xq[kk][:, :, ts(h, t_half)],
                                    start=(n_a == 0 and kk == 0),
                                    stop=(kk == n_dr - 1),
                                    perf_mode=DR,
                                )
                    for wn in range(2):
                        n = wave * 2 + wn
                        out_sb = out_pool.tile(
                            [P, t_shard], F32, name="out_sb", tag="out_sb"
                        )
                        for h in range(2):
                            nc.vector.tensor_scalar(
                                out_sb[:, ts(h, t_half)], psums[(wn, h)],
                                bias_t[(oq, n)], None,
                                mybir.AluOpType.add,
                            )
                        nc.gpsimd.dma_start(
                            out[ts(oq * n_per_oq + n, P), :], out_sb
                        )

    nc.compile()
    return nc


_NC_CACHE = {}


def _get_nc(shape_key):
    if shape_key not in _NC_CACHE:
        _NC_CACHE[shape_key] = build_nc(*shape_key)
    return _NC_CACHE[shape_key]


def make_in_maps(x, weight, bias):
    """Shard-time layout prep: transposed contiguous views per core."""
    x = np.asarray(x, dtype=np.float32)
    weight = np.asarray(weight, dtype=np.float32)
    bias = np.ascontiguousarray(np.asarray(bias, dtype=np.float32))
    tokens = x.shape[0]
    t_shard = tokens // N_CORES
    wT = np.ascontiguousarray(weight.T)
    return [
        {
            "x": np.ascontiguousarray(x[c * t_shard : (c + 1) * t_shard].T),
            "weight": wT,
            "bias": bias,
        }
        for c in range(N_CORES)
    ]


def kernel(x, weight, bias, _trace=False):
    from concourse.bass_utils import run_bass_kernel_spmd

    x = np.asarray(x, dtype=np.float32)
    tokens = x.shape[0]
    t_shard = tokens // N_CORES
    nc = _get_nc((t_shard, x.shape[1], np.asarray(weight).shape[0]))

    in_maps = make_in_maps(x, weight, bias)
    res = run_bass_kernel_spmd(
        nc, in_maps, core_ids=list(range(N_CORES)), trace=_trace
    )
    # per-core result is out_T [out_f, t_shard]; transpose during gather
    out = np.concatenate(
        [np.ascontiguousarray(r["out"].T) for r in res.results], axis=0
    )
    if _trace:
        return out, res
    return out


# revision 9
# speedup vs baseline: 1.0114x; 1.0114x over previous
"""BinaryLinear Trainium2 kernel (v5 — all-fp8 DoubleRow, hi/lo + packed).

Computes out = x @ sign(weight).T + bias for x [8192, 4096] f32,
weight [4096, 4096] f32, bias [4096] f32.

Strategy: data-parallel over the token dim across 8 NeuronCores
(1024 tokens per core, weight/bias replicated, no collectives).

v5 rationale: every matmul runs in fp8e4 DoubleRow mode (HW-measured
~228ns per K=256xM128xN512 MM here vs 253ns for a bf16 K=128 MM).
The binary weights are exact in fp8; only x quantization loses
precision, so the contraction splits into two zones:
  zone A (first N_A k-tiles): DR slots carry (e4m3(x), e4m3(residual))
    with sign(w) duplicated in both weight slots — one k-tile per MM
    at ~bf16 accuracy (residual recovers the quantization error).
  zone B (remaining k-tiles): standard DR packing, two k-tiles per MM,
    plain e4m3 x (2.64% stand-alone error, diluted by the split).
N_A=18 gives rel err 1.75% (verified exactly against the fixed
reference inputs; gate is 2e-2) at a ~365us MM floor
(16 waves x 4 x (N_A + 7) MMs x 228ns).

Per-core pipeline (host hands xT [4096i, 1024t], wT [4096i, 4096o]):
  1. x zone A: bf16 cast-load [128i, 1024t] (transient), then ScalarE
     copy -> fp8 hi slot and DVE subtract -> fp8 residual slot of
     xa [128i, 2, 1024t]. x zone B: direct SWDGE cast-load f32->fp8
     pairs [128i, 2slot, 1024t].
  2. w per o-quarter: bf16 slab cast-loads; ScalarE sign (scale=1e30
     keeps sign(0)=0; sign must see bf16, not fp8 — casting raw w to
     fp8 first would flush |w| < 2^-9 to zero) writes fp8 slots:
     zone A slot0 via ACT, slot1 duplicated via DVE copy; zone B the
     two packed k-tiles' signs.
  3. TensorE per o-pair wave: N_A zone-A DR MMs + 7 zone-B DR MMs per
     (o-tile, t-half), accumulating f32 into PSUM banks.
  4. DVE tensor_scalar adds the per-partition bias while copying
     PSUM->SBUF; SWDGE stores rows of out_T [4096, 1024].
Host: gather + transpose out_T back to [tokens, 4096].
"""

import numpy as np

import concourse.mybir as mybir
import concourse.tile as tile
from concourse import bacc
from concourse.bass import ts

P = 128
TOKENS, IN_F, OUT_F = 8192, 4096, 4096
N_CORES = 8
OQ = 1024              # output-feature quarter resident in SBUF
N_A = 18               # hi/lo k-tiles; rest packed-pair fp8

F32 = mybir.dt.float32
BF16 = mybir.dt.bfloat16
FP8 = mybir.dt.float8e4
DR = mybir.MatmulPerfMode.DoubleRow


def build_nc(t_shard=TOKENS // N_CORES, in_f=IN_F, out_f=OUT_F, repeat=1,
             n_bf=N_A):
    n_a = n_bf
    ko_tiles = in_f // P        # k tiles of 128
    n_dr = (ko_tiles - n_a) // 2
    assert n_a + 2 * n_dr == ko_tiles and n_a % 2 == 0
    oq_tiles = out_f // OQ      # resident output quarters
    n_per_oq = OQ // P          # 128-wide output tiles per quarter
    t_half = t_shard // 2       # moving-stream length per matmul

    nc = bacc.Bacc(None, target_bir_lowering=False, debug=False)

    xT = nc.dram_tensor("x", [in_f, t_shard], F32, kind="ExternalInput")
    wT = nc.dram_tensor("weight", [in_f, out_f], F32, kind="ExternalInput")
    b = nc.dram_tensor("bias", [out_f], F32, kind="ExternalInput")
    # transposed output: rows = out features, cols = this core's tokens
    out = nc.dram_tensor("out", [out_f, t_shard], F32, kind="ExternalOutput")

    with tile.TileContext(nc) as tc:
        with (
            tc.tile_pool(name="xa", bufs=max(n_a, 1)) as xa_pool,
            tc.tile_pool(name="xq", bufs=max(n_dr, 1)) as xq_pool,
            tc.tile_pool(name="wa", bufs=n_a + 6) as wa_pool,
            tc.tile_pool(name="wq", bufs=n_dr + 3) as wq_pool,
            tc.tile_pool(name="stage", bufs=6) as stage_pool,
            tc.tile_pool(name="bias", bufs=32) as bias_pool,
            tc.tile_pool(name="out_sb", bufs=4) as out_pool,
            tc.tile_pool(name="ps", bufs=8, space="PSUM") as psum_pool,
        ):
          for _rep in range(repeat):

            def emit_w_a(oq, k):
                """Zone-A w tile [128i, 2, OQ] fp8, sign duplicated."""
                wa_k = wa_pool.tile(
                    [P, OQ // P, 2, P], FP8, name=f"wa_{oq}_{k}", tag="wa"
                )
                slab = stage_pool.tile(
                    [P, OQ], BF16, name=f"was_{oq}_{k}", tag="stage"
                )
                nc.gpsimd.dma_start(slab, wT[ts(k, P), ts(oq, OQ)])
                nc.scalar.activation(
                    wa_k[:, :, 0, :], slab,
                    mybir.ActivationFunctionType.Sign, scale=1.0e30,
                )
                nc.vector.tensor_copy(wa_k[:, :, 1, :], wa_k[:, :, 0, :])
                return wa_k

            def emit_w_dr(oq, kk):
                """Zone-B w pair-tile [128i, 2, OQ] fp8 (packed k-tiles)."""
                wq_k = wq_pool.tile(
                    [P, OQ // P, 2, P], FP8, name=f"wq_{oq}_{kk}", tag="wq"
                )
                for s in range(2):
                    k = n_a + 2 * kk + s
                    slab = stage_pool.tile(
                        [P, OQ], BF16, name=f"ws_{oq}_{kk}_{s}", tag="stage"
                    )
                    nc.gpsimd.dma_start(slab, wT[ts(k, P), ts(oq, OQ)])
                    nc.scalar.activation(
                        wq_k[:, :, s, :], slab,
                        mybir.ActivationFunctionType.Sign, scale=1.0e30,
                    )
                return wq_k

            def emit_x_a(k):
                """Zone-A x tile [128i, 2, 1024t] fp8: (hi, residual)."""
                xb_k = stage_pool.tile(
                    [P, t_shard], BF16, name=f"xb_{k}", tag="stage"
                )
                nc.gpsimd.dma_start(xb_k, xT[ts(k, P), :])
                xa_k = xa_pool.tile(
                    [P, 2, t_shard], FP8, name=f"xa_{k}", tag="xa"
                )
                nc.scalar.activation(
                    xa_k[:, 0, :], xb_k,
                    mybir.ActivationFunctionType.Copy,
                )
                nc.vector.tensor_tensor(
                    xa_k[:, 1, :], xb_k, xa_k[:, 0, :],
                    mybir.AluOpType.subtract,
                )
                return xa_k

            def emit_x_dr(kk):
                xq_k = xq_pool.tile(
                    [P, 2, t_shard], FP8, name=f"xq_{kk}", tag="xq"
                )
                src = xT[ts(n_a // 2 + kk, 2 * P), :].rearrange(
                    "(s p) t -> p s t", s=2
                )
                nc.gpsimd.dma_start(xq_k, src)
                return xq_k

            # ---- head: x and first-quarter w interleaved in k order so
            # the k=0 matmuls start after a few small loads.
            xa, xq = {}, {}
            wa = {0: [emit_w_a(0, 0)]}
            wq = {0: []}
            for k in range(ko_tiles):
                if k < n_a:
                    xa[k] = emit_x_a(k)
                elif (k - n_a) % 2 == 0:
                    xq[(k - n_a) // 2] = emit_x_dr((k - n_a) // 2)
                if k + 1 < n_a:
                    wa[0].append(emit_w_a(0, k + 1))
                elif k + 1 >= n_a and (k + 1 - n_a) % 2 == 0 and k + 1 < ko_tiles:
                    wq[0].append(emit_w_dr(0, (k + 1 - n_a) // 2))

            bias_t = {}
            for oq in range(oq_tiles):
                for n in range(n_per_oq):
                    bias_n = bias_pool.tile(
                        [P, 1], F32, name=f"b_{oq}_{n}", tag="bias"
                    )
                    nc.gpsimd.dma_start(bias_n, b[ts(oq * n_per_oq + n, P), None])
                    bias_t[(oq, n)] = bias_n

            # ---- main loop: o-quarters, waves of 2 output tiles
            for oq in range(oq_tiles):
                if oq + 1 < oq_tiles:
                    wa[oq + 1] = [emit_w_a(oq + 1, k) for k in range(n_a)]
                    wq[oq + 1] = [emit_w_dr(oq + 1, kk) for kk in range(n_dr)]
                was = wa.pop(oq)
                wqs = wq.pop(oq)

                for wave in range(n_per_oq // 2):
                    psums = {}
                    for wn in range(2):
                        for h in range(2):
                            psums[(wn, h)] = psum_pool.tile(
                                [P, t_half], F32,
                                name=f"ps_{oq}_{wave}_{wn}_{h}", tag="ps",
                            )
                    for k in range(n_a):
                        for wn in range(2):
                            n = wave * 2 + wn
                            for h in range(2):
                                nc.tensor.matmul(
                                    psums[(wn, h)],
                                    was[k][:, n, :, :],
                                    xa[k][:, :, ts(h, t_half)],
                                    start=(k == 0),
                                    stop=(n_dr == 0 and k == n_a - 1),
                                    perf_mode=DR,
                                )
                    for kk in range(n_dr):
                        for wn in range(2):
                            n = wave * 2 + wn
                            for h in range(2):
                                nc.tensor.matmul(
                                    psums[(wn, h)],
                                    wqs[kk][:, n, :, :],
                                    xq[kk][:, :, ts(h, t_half)],
                                    start=(n_a == 0 and kk == 0),
                                    stop=(kk == n_dr - 1),
                                    perf_mode=DR,
                                )
                    for wn in range(2):
                        n = wave * 2 + wn
                        out_sb = out_pool.tile(
                            [P, t_shard], F32, name="out_sb", tag="out_sb"
                        )
                        for h in range(2):
                            nc.vector.tensor_scalar(
                                out_sb[:, ts(h, t_half)], psums[(wn, h)],
                                bias_t[(oq, n)], None,
                                mybir.AluOpType.add,
                            )
                        nc.gpsimd.dma_start(
                            out[ts(oq * n_per_oq + n, P), :], out_sb
                        )

    nc.compile()
    return nc


_NC_CACHE = {}


def _get_nc(shape_key):
    if shape_key not in _NC_CACHE:
        _NC_CACHE[shape_key] = build_nc(*shape_key)
    return _NC_CACHE[shape_key]


def make_in_maps(x, weight, bias):
    """Shard-time layout prep: transposed contiguous views per core."""
    x = np.asarray(x, dtype=np.float32)
    weight = np.asarray(weight, dtype=np.float32)
    bias = np.ascontiguousarray(np.asarray(bias, dtype=np.float32))
    tokens = x.shape[0]
    t_shard = tokens // N_CORES
    wT = np.ascontiguousarray(weight.T)
    return [
        {
            "x": np.ascontiguousarray(x[c * t_shard : (c + 1) * t_shard].T),
            "weight": wT,
            "bias": bias,
        }
        for c in range(N_CORES)
    ]


def kernel(x, weight, bias, _trace=False):
    from concourse.bass_utils import run_bass_kernel_spmd

    x = np.asarray(x, dtype=np.float32)
    tokens = x.shape[0]
    t_shard = tokens // N_CORES
    nc = _get_nc((t_shard, x.shape[1], np.asarray(weight).shape[0]))

    in_maps = make_in_maps(x, weight, bias)
    res = run_bass_kernel_spmd(
        nc, in_maps, core_ids=list(range(N_CORES)), trace=_trace
    )
    # per-core result is out_T [out_f, t_shard]; transpose during gather
    out = np.concatenate(
        [np.ascontiguousarray(r["out"].T) for r in res.results], axis=0
    )
    if _trace:
        return out, res
    return out


# revision 10
# speedup vs baseline: 1.0131x; 1.0017x over previous
"""BinaryLinear Trainium2 kernel (v4 — bf16 + fp8 DoubleRow hybrid).

Computes out = x @ sign(weight).T + bias for x [8192, 4096] f32,
weight [4096, 4096] f32, bias [4096] f32.

Strategy: data-parallel over the token dim across 8 NeuronCores
(1024 tokens per core, weight/bias replicated, no collectives).

v4 rationale: v3 is PE-bound at ~553us (2048 bf16 MMs of N=512 at
the observed ~2.0 GHz PE clock, transpose-free DMA pipeline). The
binary weights are exact in fp8, so the only fp8 quantization loss is
on x. HW-measured here: a DoubleRow fp8 MM (K=256 logical) costs
~228ns vs 253ns for a bf16 MM (K=128) — 2.2x work rate. A pure-fp8
kernel fails accuracy (e4m3 x => rel err 2.64% > 2e-2 gate), so the
contraction is split: the first N_BF k-tiles run bf16, the remaining
32-N_BF run as e4m3 DoubleRow pairs. N_BF=18 gives rel err 1.76%
(verified exactly against the fixed reference inputs) and cuts MM
time to ~394us.

Per-core pipeline (host hands xT [4096i, 1024t], wT [4096i, 4096o]):
  1. x: k-tiles 0..N_BF-1 as bf16 [128i, 1024t] SWDGE cast-loads;
     k-tile pairs above N_BF as fp8e4 [128i, 2slot, 1024t] direct
     SWDGE cast-loads (slot s = k-tile N_BF+2kk+s).
  2. w per o-quarter: bf16 k-slabs [128i, 1024o] cast-loads with
     sign() in place on ScalarE (scale=1e30 keeps sign(0)=0); for DR
     pairs the sign output is written as fp8e4 into wq [128i, 2,
     1024o] slots (sign must run on bf16 input: casting raw w to fp8
     first would flush |w| < 2^-9 to zero and corrupt signs).
  3. TensorE per o-pair wave: k-loop 0..N_BF-1 bf16 MMs, then kk-loop
     DoubleRow MMs (lhsT [128,2,128] fp8, rhs [128,2,512] fp8),
     all accumulating f32 into the same PSUM banks.
  4. DVE tensor_scalar adds the per-partition bias while copying
     PSUM->SBUF; SWDGE stores rows of out_T [4096, 1024].
Host: gather + transpose out_T back to [tokens, 4096].
"""

import numpy as np

import concourse.mybir as mybir
import concourse.tile as tile
from concourse import bacc
from concourse.bass import ts

P = 128
TOKENS, IN_F, OUT_F = 8192, 4096, 4096
N_CORES = 8
OQ = 1024              # output-feature quarter resident in SBUF
N_BF = 18              # k-tiles computed in bf16; rest fp8 DoubleRow

F32 = mybir.dt.float32
BF16 = mybir.dt.bfloat16
FP8 = mybir.dt.float8e4
DR = mybir.MatmulPerfMode.DoubleRow


def build_nc(t_shard=TOKENS // N_CORES, in_f=IN_F, out_f=OUT_F, repeat=1,
             n_bf=N_BF):
    ko_tiles = in_f // P        # k tiles of 128
    n_dr = (ko_tiles - n_bf) // 2
    assert n_bf + 2 * n_dr == ko_tiles
    oq_tiles = out_f // OQ      # resident output quarters
    n_per_oq = OQ // P          # 128-wide output tiles per quarter
    t_half = t_shard // 2       # moving-stream length per matmul

    nc = bacc.Bacc(None, target_bir_lowering=False, debug=False)

    xT = nc.dram_tensor("x", [in_f, t_shard], F32, kind="ExternalInput")
    wT = nc.dram_tensor("weight", [in_f, out_f], F32, kind="ExternalInput")
    b = nc.dram_tensor("bias", [out_f], F32, kind="ExternalInput")
    # transposed output: rows = out features, cols = this core's tokens
    out = nc.dram_tensor("out", [out_f, t_shard], F32, kind="ExternalOutput")

    with tile.TileContext(nc) as tc:
        with (
            tc.tile_pool(name="xt", bufs=max(n_bf, 1)) as xt_pool,
            tc.tile_pool(name="xq", bufs=max(n_dr, 1)) as xq_pool,
            tc.tile_pool(name="wt", bufs=n_bf + 6) as wt_pool,
            tc.tile_pool(name="wq", bufs=n_dr + 3) as wq_pool,
            tc.tile_pool(name="stage", bufs=4) as stage_pool,
            tc.tile_pool(name="bias", bufs=32) as bias_pool,
            tc.tile_pool(name="out_sb", bufs=4) as out_pool,
            tc.tile_pool(name="ps", bufs=8, space="PSUM") as psum_pool,
        ):
          for _rep in range(repeat):

            def emit_w_bf(oq, k):
                """Signed bf16 w slab [128i, OQ] for (o-quarter, k-tile)."""
                wt_k = wt_pool.tile([P, OQ], BF16, name=f"w_{oq}_{k}", tag="wt")
                nc.gpsimd.dma_start(wt_k, wT[ts(k, P), ts(oq, OQ)])
                nc.scalar.activation(
                    wt_k, wt_k, mybir.ActivationFunctionType.Sign,
                    scale=1.0e30,
                )
                return wt_k

            def emit_w_dr(oq, kk):
                """Signed fp8 w pair-tile [128i, 2, OQ] for DR step kk."""
                wq_k = wq_pool.tile(
                    [P, 2, OQ], FP8, name=f"wq_{oq}_{kk}", tag="wq"
                )
                for s in range(2):
                    k = n_bf + 2 * kk + s
                    slab = stage_pool.tile(
                        [P, OQ], BF16, name=f"ws_{oq}_{kk}_{s}", tag="stage"
                    )
                    nc.gpsimd.dma_start(slab, wT[ts(k, P), ts(oq, OQ)])
                    nc.scalar.activation(
                        wq_k[:, s, :], slab,
                        mybir.ActivationFunctionType.Sign, scale=1.0e30,
                    )
                return wq_k

            def emit_x_bf(k):
                xt_k = xt_pool.tile([P, t_shard], BF16, name=f"x_{k}", tag="xt")
                nc.gpsimd.dma_start(xt_k, xT[ts(k, P), :])
                return xt_k

            def emit_x_dr(kk):
                xq_k = xq_pool.tile(
                    [P, 2, t_shard], FP8, name=f"xq_{kk}", tag="xq"
                )
                src = xT[ts(n_bf // 2 + kk, 2 * P), :].rearrange(
                    "(s p) t -> p s t", s=2
                )
                nc.gpsimd.dma_start(xq_k, src)
                return xq_k

            # ---- head: x and first-quarter w interleaved in k order so
            # the k=0 matmuls start after two small loads.
            xt, xq = {}, {}
            wslab = {0: [emit_w_bf(0, 0)]}
            wqtiles = {0: []}
            for k in range(ko_tiles):
                if k < n_bf:
                    xt[k] = emit_x_bf(k)
                elif (k - n_bf) % 2 == 0:
                    xq[(k - n_bf) // 2] = emit_x_dr((k - n_bf) // 2)
                if k + 1 < n_bf:
                    wslab[0].append(emit_w_bf(0, k + 1))
                elif k + 1 >= n_bf and (k + 1 - n_bf) % 2 == 0 and k + 1 < ko_tiles:
                    wqtiles[0].append(emit_w_dr(0, (k + 1 - n_bf) // 2))

            bias_t = {}
            for oq in range(oq_tiles):
                for n in range(n_per_oq):
                    bias_n = bias_pool.tile(
                        [P, 1], F32, name=f"b_{oq}_{n}", tag="bias"
                    )
                    nc.gpsimd.dma_start(bias_n, b[ts(oq * n_per_oq + n, P), None])
                    bias_t[(oq, n)] = bias_n

            # ---- main loop: o-quarters, waves of 2 output tiles
            for oq in range(oq_tiles):
                if oq + 1 < oq_tiles:
                    wslab[oq + 1] = [
                        emit_w_bf(oq + 1, k) for k in range(n_bf)
                    ]
                    wqtiles[oq + 1] = [
                        emit_w_dr(oq + 1, kk) for kk in range(n_dr)
                    ]
                slabs = wslab.pop(oq)
                wqs = wqtiles.pop(oq)

                for wave in range(n_per_oq // 2):
                    psums = {}
                    for wn in range(2):
                        for h in range(2):
                            psums[(wn, h)] = psum_pool.tile(
                                [P, t_half], F32,
                                name=f"ps_{oq}_{wave}_{wn}_{h}", tag="ps",
                            )
                    for k in range(n_bf):
                        for wn in range(2):
                            n = wave * 2 + wn
                            for h in range(2):
                                nc.tensor.matmul(
                                    psums[(wn, h)],
                                    slabs[k][:, ts(n, P)],
                                    xt[k][:, ts(h, t_half)],
                                    start=(k == 0),
                                    stop=(n_dr == 0 and k == n_bf - 1),
                                )
                    for kk in range(n_dr):
                        for wn in range(2):
                            n = wave * 2 + wn
                            for h in range(2):
                                nc.tensor.matmul(
                                    psums[(wn, h)],
                                    wqs[kk][:, :, ts(n, P)],
                                    xq[kk][:, :, ts(h, t_half)],
                                    start=(n_bf == 0 and kk == 0),
                                    stop=(kk == n_dr - 1),
                                    perf_mode=DR,
                                )
                    for wn in range(2):
                        n = wave * 2 + wn
                        out_sb = out_pool.tile(
                            [P, t_shard], F32, name="out_sb", tag="out_sb"
                        )
                        for h in range(2):
                            nc.vector.tensor_scalar(
                                out_sb[:, ts(h, t_half)], psums[(wn, h)],
                                bias_t[(oq, n)], None,
                                mybir.AluOpType.add,
                            )
                        nc.gpsimd.dma_start(
                            out[ts(oq * n_per_oq + n, P), :], out_sb
                        )

    nc.compile()
    return nc


_NC_CACHE = {}


def _get_nc(shape_key):
    if shape_key not in _NC_CACHE:
        _NC_CACHE[shape_key] = build_nc(*shape_key)
    return _NC_CACHE[shape_key]


def make_in_maps(x, weight, bias):
    """Shard-time layout prep: transposed contiguous views per core."""
    x = np.asarray(x, dtype=np.float32)
    weight = np.asarray(weight, dtype=np.float32)
    bias = np.ascontiguousarray(np.asarray(bias, dtype=np.float32))
    tokens = x.shape[0]
    t_shard = tokens // N_CORES
    wT = np.ascontiguousarray(weight.T)
    return [
        {
            "x": np.ascontiguousarray(x[c * t_shard : (c + 1) * t_shard].T),
            "weight": wT,
            "bias": bias,
        }
        for c in range(N_CORES)
    ]


def kernel(x, weight, bias, _trace=False):
    from concourse.bass_utils import run_bass_kernel_spmd

    x = np.asarray(x, dtype=np.float32)
    tokens = x.shape[0]
    t_shard = tokens // N_CORES
    nc = _get_nc((t_shard, x.shape[1], np.asarray(weight).shape[0]))

    in_maps = make_in_maps(x, weight, bias)
    res = run_bass_kernel_spmd(
        nc, in_maps, core_ids=list(range(N_CORES)), trace=_trace
    )
    # per-core result is out_T [out_f, t_shard]; transpose during gather
    out = np.concatenate(
        [np.ascontiguousarray(r["out"].T) for r in res.results], axis=0
    )
    if _trace:
        return out, res
    return out
